# revision 22
# baseline (speedup 1.0000x reference)
"""Trainium2 Bass kernel for nn_AdaptiveSampler (sparse grid_sample attention).

Strategy v3 (data-parallel over batch, 8 cores x 4 batch items each):
  - Host: features channels-last [B*H*W, C] bf16 (2KB rows). All gather
    indices depend only on keypoint_coords, so the host precomputes them
    in the gpsimd wrapped int16 layout, plus the per-column seed bilinear
    weights.
  - Device per core:
      seed: ONE transpose-gather of 2x2 corner cells as 2-cell rows
            (channel-major [128c, (x,q), (y,jb)]), then DVE multiply by
            per-column corner weights (partition-broadcast, 2x bf16 rate)
            and two halving adds -> seed lands directly channel-major,
            feeding the MLP matmuls with no PE transposes.
      patch: one 3x3-cell gather per batch item, keypoint-major
            ([128j, 3y, 3x*1024c]) for the fuse stage.
      MLPs: PE matmuls; offsets/attention transposed back per-b (PE).
      fuse: 9 diagonal-matmuls per batch item on PE: stationary
            diag(w9[:,xy]) built on the Scalar engine (activation with
            per-partition scale), moving = patch x-slices, accumulated in
            PSUM f32 -> output lands keypoint-major [j, C]; direct DMA out.
"""

import os
import sys
from contextlib import ExitStack

import numpy as np

sys.path.insert(0, "/opt/trn_rl_repo")

import ml_dtypes

import concourse.bass as bass
import concourse.tile as tile
from concourse import bacc, mybir

F32 = mybir.dt.float32
BF16 = mybir.dt.bfloat16
I16 = mybir.dt.int16

ALU = mybir.AluOpType
ACT = mybir.ActivationFunctionType
AX = mybir.AxisListType

B = 4          # batch items per core
C = 1024       # channels
H = W = 64
HW = H * W     # 4096 cells per batch item
J = 128        # keypoints
NP = 4         # sample points per keypoint
Q = C // 128   # 8 channel chunks
TWO23 = float(2 ** 23)


def _floor(nc, pool, src, shape, tag):
    """floor(src) on DVE via round-to-nearest + correction. Returns tile."""
    rnd = pool.tile(list(shape), F32, tag=f"floor_rnd_{tag}")
    nc.vector.tensor_scalar(rnd[:], src, TWO23, TWO23, ALU.add, ALU.subtract)
    flo = pool.tile(list(shape), F32, tag=f"floor_out_{tag}")
    nc.vector.tensor_tensor(flo[:], src, rnd[:], ALU.is_lt)
    nc.vector.tensor_tensor(flo[:], rnd[:], flo[:], ALU.subtract)
    return flo


def build_nc():
    nc = bacc.Bacc()

    feat = nc.declare_dram_parameter("feat", [B * HW, C], BF16, isOutput=False)
    # seed idx [128, 64] ++ per-b patch idx [128, 24] each
    wsg = nc.declare_dram_parameter("wsg", [128, 2 * 2 * J * B], BF16,
                                    isOutput=False)
    # aligned (4KB-row) matmul weights: [w1o Q*128 | w1a Q*128]
    w1pk_d = nc.declare_dram_parameter("w1pk", [128, 2048], BF16, isOutput=False)
    # bf16 pack: [identb 128 | w2o 8 | w2a 4 | idx 160]
    bpack = nc.declare_dram_parameter("bpack", [128, 300], BF16, isOutput=False)
    # f32 pack: [coef 16 | b1o | b1a | posc 4 | b2o col | b2a col]
    fpack = nc.declare_dram_parameter("fpack", [128, 24], F32, isOutput=False)
    out = nc.declare_dram_parameter("out", [B * J, C], BF16, isOutput=True)

    # Overlapping row views of feat. 3-cell rows (patch): max start 16381.
    # 2-cell rows (seed): max start 16382. Read end == tensor end exactly.
    feat_ov3 = bass.AP(feat[:].tensor, 0, [[C, B * HW - 2], [1, 3 * C]])
    feat_ov2 = bass.AP(feat[:].tensor, 0, [[C, B * HW - 1], [1, 2 * C]])

    with ExitStack() as ctx:
        tc = ctx.enter_context(tile.TileContext(nc))
        cons = ctx.enter_context(tc.tile_pool(name="cons", bufs=1))
        gp = ctx.enter_context(tc.tile_pool(name="gpool", bufs=1))
        a = ctx.enter_context(tc.tile_pool(name="work", bufs=1))
        dgp = ctx.enter_context(tc.tile_pool(name="diag", bufs=2))
        ps = ctx.enter_context(tc.tile_pool(name="psT", bufs=2, space="PSUM"))
        pmm = ctx.enter_context(tc.tile_pool(name="psMM", bufs=2, space="PSUM"))
        pfu = ctx.enter_context(tc.tile_pool(name="psFU", bufs=3, space="PSUM"))

        # ---------------- constants ----------------
        # ONE packed const DMA carries the gather indices (bitcast bf16
        # columns) so the gathers wait on a single large-packet transfer.
        bpk = cons.tile([128, 300], BF16, tag="bpk")
        nc.scalar.dma_start(out=bpk[:], in_=bpack[:])
        idxg_sb = bpk[:, 140:300].bitcast(I16)

        # ---------------- seed gather (channel-major, 4 chunks) -----------
        # chunk h: y = h//2, jb half = h%2 (256 idxs each)
        G2h = [None] * 4
        for h in (0, 2, 1, 3):
            g2 = gp.tile([128, 16, 256], BF16, tag=f"G2{h}")
            nc.gpsimd.dma_gather(
                g2[:],
                feat_ov2,
                idxg_sb[:, 16 * h : 16 * h + 16],
                num_idxs=256,
                num_idxs_reg=256,
                elem_size=2 * C,
                elem_step=C,
                transpose=True,
            )
            G2h[h] = g2

        # ---------------- patch gathers (keypoint-major, one per b) -------
        Gt = []
        for b in range(B):
            g = gp.tile([128, 3, 3 * C], BF16, tag=f"G{b}")
            nc.gpsimd.dma_gather(
                g[:],
                feat_ov3,
                idxg_sb[:, 64 + b * 24 : 64 + (b + 1) * 24],
                num_idxs=3 * J,
                num_idxs_reg=3 * J,
                elem_size=3 * C,
                elem_step=C,
                transpose=False,
            )
            Gt.append(g)

        # ---------------- remaining constants (overlap the gathers) -------
        fpk = cons.tile([128, 24], F32, tag="fpk")
        nc.sync.dma_start(out=fpk[:], in_=fpack[:])
        wsg_sb = cons.tile([128, 2, 2, J * B], BF16, tag="wsg")
        nc.sync.dma_start(
            out=wsg_sb[:],
            in_=wsg[:].rearrange("p (x y i) -> p x y i", x=2, y=2),
        )

        w1pk = cons.tile([128, 2048], BF16, tag="w1pk")
        nc.sync.dma_start(out=w1pk[:], in_=w1pk_d[:])
        w1o_sb = w1pk[:, 0:1024].rearrange("p (q m) -> p q m", q=Q)
        w1a_sb = w1pk[:, 1024:2048].rearrange("p (q m) -> p q m", q=Q)
        idb_sb = bpk[:, 0:128]
        w2o_sb = bpk[:, 128:136]
        w2a_sb = bpk[:, 136:140]
        b1o_sb = fpk[:, 16:17]
        b1a_sb = fpk[:, 17:18]
        posc_sb = fpk[:, 18:22]
        b2o_sb = fpk[0:8, 22:23]
        b2a_sb = fpk[0:4, 23:24]

        ixv = fpk[:, 0:4]    # [J, B] pixel x coords
        iyv = fpk[:, 4:8]
        bxv = fpk[:, 8:12]   # patch x base (f32 integer-valued)
        byv = fpk[:, 12:16]

        # ---------------- seed combine (DVE, 2x-rate contiguous ops) ------
        seed = a.tile([128, Q, J * B], BF16)

        def seed_half(jh):
            for h in (jh, 2 + jh):
                y = h // 2
                g2 = G2h[h]
                # [128, q8, 256jb] per x-half *= wsg[x, y(h), jb-slice]
                g2v = g2[:].rearrange("p (x q) i -> p x q i", x=2)
                for x in range(2):
                    nc.vector.tensor_tensor(
                        g2v[:, x, :, :],
                        g2v[:, x, :, :],
                        wsg_sb[:, x, y, 256 * jh : 256 * jh + 256]
                        .unsqueeze(1)
                        .to_broadcast((128, Q, 256)),
                        ALU.mult,
                    )
                # x-add: first half += second half (contiguous)
                g2f = g2[:].rearrange("p e i -> p (e i)")
                nc.vector.tensor_tensor(
                    g2f[0:128, 0 : Q * 256],
                    g2f[0:128, 0 : Q * 256],
                    g2f[0:128, Q * 256 : 2 * Q * 256],
                    ALU.add,
                )
            # y-add into contiguous seed tile for this jb half
            nc.vector.tensor_tensor(
                seed[:, :, 256 * jh : 256 * jh + 256],
                G2h[jh][:, 0:Q, :],
                G2h[2 + jh][:, 0:Q, :],
                ALU.add,
            )

        with nc.allow_low_precision("bf16 grid-sample compute"):
            seed_half(0)
            seed_half(1)

        # ---------------- MLP + weights + fuse, per jb-half pipeline -----
        # half jh covers jb columns [256*jh, 256*jh+256) = batch b=2jh, 2jh+1
        offT = a.tile([J, B, 8], F32)
        attT = a.tile([J, B, 4], F32)

        def mlp_chain(jh):
            sl = slice(256 * jh, 256 * jh + 256)

            def head(w1_sb, b1_sb, w2_sb, b2_sb, m2, name):
                hps = pmm.tile([128, 256], F32, tag="mlp")
                for q in range(Q):
                    nc.tensor.matmul(
                        hps[:], w1_sb[:, q, :], seed[:, q, sl],
                        start=(q == 0), stop=(q == Q - 1),
                    )
                h_sb = a.tile([128, 256], BF16, tag=f"hsb_{name}{jh}")
                nc.scalar.activation(h_sb[:], hps[:], ACT.Relu, bias=b1_sb)
                ps2 = pmm.tile([m2, 256], F32, tag="mlp")
                nc.tensor.matmul(ps2[:], w2_sb, h_sb[:], start=True, stop=True)
                o2 = a.tile([m2, 256], BF16, tag=f"o2_{name}{jh}")
                nc.scalar.activation(o2[:], ps2[:], ACT.Identity, bias=b2_sb)
                return o2

            off2 = head(w1o_sb, b1o_sb, w2o_sb, b2o_sb, 8, "off")
            att2 = head(w1a_sb, b1a_sb, w2a_sb, b2a_sb, 4, "att")
            for bl in range(2):
                b = 2 * jh + bl
                pso = ps.tile([128, 8], BF16, tag="tp")
                nc.tensor.transpose(
                    pso[:, 0:8], off2[:, bl * J : (bl + 1) * J],
                    idb_sb[0:8, 0:8],
                )
                nc.scalar.copy(offT[:, b, :], pso[:, 0:8])
                psa = ps.tile([128, 4], BF16, tag="tp")
                nc.tensor.transpose(
                    psa[:, 0:4], att2[:, bl * J : (bl + 1) * J],
                    idb_sb[0:4, 0:4],
                )
                nc.scalar.copy(attT[:, b, :], psa[:, 0:4])

        def axis_select(pc, base, tagp):
            """Position-select weights [J, 2, NP, 3]:
            w0*(pos==d) + w1*(pos==d+1), d = floor(pc) - base."""
            c0 = _floor(nc, a, pc[:], (J, 2, NP), tagp)
            w1t = a.tile([J, 2, NP], F32, tag=f"{tagp}_w1")
            nc.vector.tensor_tensor(w1t[:], pc[:], c0[:], ALU.subtract)
            w0t = a.tile([J, 2, NP], F32, tag=f"{tagp}_w0")
            nc.vector.tensor_scalar(w0t[:], w1t[:], -1.0, 1.0, ALU.mult, ALU.add)
            d = a.tile([J, 2, NP], F32, tag=f"{tagp}_d")
            nc.vector.tensor_tensor(
                d[:], c0[:], base.unsqueeze(2).to_broadcast((J, 2, NP)),
                ALU.subtract,
            )
            d1 = a.tile([J, 2, NP], F32, tag=f"{tagp}_d1")
            nc.vector.tensor_scalar_add(d1[:], d[:], 1.0)
            posb = (
                posc_sb[:, 0:3]
                .unsqueeze(1)
                .unsqueeze(2)
                .to_broadcast((J, 2, NP, 3))
            )
            sel = a.tile([J, 2, NP, 3], F32, tag=f"{tagp}_sel")
            eq = a.tile([J, 2, NP, 3], F32, tag=f"{tagp}_eq")
            nc.vector.tensor_tensor(
                eq[:], d[:].unsqueeze(3).to_broadcast((J, 2, NP, 3)), posb,
                ALU.is_equal,
            )
            nc.vector.tensor_tensor(
                sel[:], eq[:], w0t[:].unsqueeze(3).to_broadcast((J, 2, NP, 3)),
                ALU.mult,
            )
            nc.vector.tensor_tensor(
                eq[:], d1[:].unsqueeze(3).to_broadcast((J, 2, NP, 3)), posb,
                ALU.is_equal,
            )
            nc.vector.tensor_tensor(
                eq[:], eq[:], w1t[:].unsqueeze(3).to_broadcast((J, 2, NP, 3)),
                ALU.mult,
            )
            nc.vector.tensor_tensor(sel[:], sel[:], eq[:], ALU.add)
            return sel

        def weights_pair(jh):
            """Fuse weights for batch pair (2jh, 2jh+1) -> w9b bf16 [J,2,9]."""
            bsl = slice(2 * jh, 2 * jh + 2)
            px = a.tile([J, 2, NP], F32, tag=f"px{jh}")
            nc.vector.tensor_tensor(
                px[:],
                ixv[:, bsl].unsqueeze(2).to_broadcast((J, 2, NP)),
                offT[:, bsl, 0:NP],
                ALU.add,
            )
            py = a.tile([J, 2, NP], F32, tag=f"py{jh}")
            nc.vector.tensor_tensor(
                py[:],
                iyv[:, bsl].unsqueeze(2).to_broadcast((J, 2, NP)),
                offT[:, bsl, NP : 2 * NP],
                ALU.add,
            )
            amax = a.tile([J, 2, 1], F32, tag=f"amax{jh}")
            nc.vector.tensor_reduce(amax[:], attT[:, bsl, :], AX.X, ALU.max)
            ae = a.tile([J, 2, NP], F32, tag=f"ae{jh}")
            nc.vector.tensor_tensor(
                ae[:], attT[:, bsl, :], amax[:].to_broadcast((J, 2, NP)),
                ALU.subtract,
            )
            nc.scalar.activation(ae[:], ae[:], ACT.Exp)
            asum = a.tile([J, 2, 1], F32, tag=f"asum{jh}")
            nc.vector.tensor_reduce(asum[:], ae[:], AX.X, ALU.add)
            nc.vector.reciprocal(asum[:], asum[:])
            attw = a.tile([J, 2, NP], F32, tag=f"attw{jh}")
            nc.vector.tensor_tensor(
                attw[:], ae[:], asum[:].to_broadcast((J, 2, NP)), ALU.mult
            )
            wxsel = axis_select(px, bxv[:, bsl], f"sx{jh}")
            wysel = axis_select(py, byv[:, bsl], f"sy{jh}")
            ty = a.tile([J, 2, NP, 3], F32, tag=f"ty{jh}")
            nc.vector.tensor_tensor(
                ty[:], wysel[:],
                attw[:].unsqueeze(3).to_broadcast((J, 2, NP, 3)),
                ALU.mult,
            )
            w9 = a.tile([J, 2, 3, 3], F32, tag=f"w9{jh}")
            tmp9 = a.tile([J, 2, 3, 3], F32, tag=f"tmp9{jh}")
            for n in range(NP):
                dst = (w9 if n == 0 else tmp9)
                nc.vector.tensor_tensor(
                    dst[:],
                    ty[:, :, n, :].unsqueeze(3).to_broadcast((J, 2, 3, 3)),
                    wxsel[:, :, n, :].unsqueeze(2).to_broadcast((J, 2, 3, 3)),
                    ALU.mult,
                )
                if n > 0:
                    nc.vector.tensor_tensor(w9[:], w9[:], tmp9[:], ALU.add)
            w9b = a.tile([J, 2, 9], BF16, tag=f"w9b{jh}")
            nc.vector.tensor_copy(
                w9b[:], w9[:].rearrange("j b y x -> j b (y x)")
            )
            return w9, w9b

        def fuse_b(b, w9, w9b, bl, use_scalar):
            dgs = []
            for k in range(9):
                dg = dgp.tile([128, 128], BF16, tag=f"dg{k}",
                              padded_shape=[128, 1024])
                if use_scalar:
                    nc.scalar.activation(
                        dg[:], idb_sb, ACT.Identity,
                        scale=w9[:, bl, k // 3, k % 3 : k % 3 + 1],
                    )
                else:
                    nc.gpsimd.tensor_tensor(
                        dg[:],
                        idb_sb,
                        w9b[:, bl, k : k + 1].to_broadcast((128, 128)),
                        ALU.mult,
                    )
                dgs.append(dg)
            fo = a.tile([128, C], BF16, tag=f"fo{b}")
            acc0 = pfu.tile([128, 512], F32, tag="facc")
            acc1 = pfu.tile([128, 512], F32, tag="facc")
            k = 0
            for y in range(3):
                for x in range(3):
                    for acc, hh in ((acc0, 0), (acc1, 1)):
                        nc.tensor.matmul(
                            acc[:],
                            dgs[k][:],
                            Gt[b][
                                :, y, x * C + hh * 512 : x * C + hh * 512 + 512
                            ],
                            start=(k == 0),
                            stop=(k == 8),
                            skip_group_check=True,
                        )
                    k += 1
            nc.scalar.copy(fo[:, 0:512], acc0[:])
            nc.scalar.copy(fo[:, 512:1024], acc1[:])
            nc.sync.dma_start(out=out[b * J : (b + 1) * J, :], in_=fo[:])

        mlp_chain(0)
        mlp_chain(1)
        w90, w9b0 = weights_pair(0)
        w91, w9b1 = weights_pair(1)
        # pair-0 diags on the idle Scalar engine (activation w/ scale);
        # pair-1 on gpsimd (free once the gathers drain)
        fuse_b(0, w90, w9b0, 0, True)
        fuse_b(1, w90, w9b0, 1, True)
        fuse_b(2, w91, w9b1, 0, False)
        fuse_b(3, w91, w9b1, 1, False)

    nc.finalize()
    return nc


def prepare_in_maps(features, keypoint_coords, w_off1, b_off1, w_off2, b_off2,
                    w_att1, b_att1, w_att2, b_att2, n_cores=8):
    bf = ml_dtypes.bfloat16
    f32 = np.float32

    def w1t(w):  # [128, C] -> [128 k_local, Q, 128 m] bf16
        return np.ascontiguousarray(
            w.T.reshape(Q, 128, 128).transpose(1, 0, 2).astype(bf)
        )

    def wrap(flat):  # [N] int16 -> [128, N//16] gpsimd wrapped layout
        n = flat.shape[0]
        return np.tile(flat.reshape(n // 16, 16).T, (8, 1))

    w1o_h = w1t(np.asarray(w_off1, f32))
    w1a_h = w1t(np.asarray(w_att1, f32))
    w2o_h = np.ascontiguousarray(
        np.concatenate([w_off2[0::2], w_off2[1::2]], 0).T.astype(bf)
    )
    w2a_h = np.ascontiguousarray(np.asarray(w_att2, f32).T.astype(bf))
    w1pk_h = np.empty((128, 2048), bf)
    w1pk_h[:, 0:1024] = w1o_h.reshape(128, 1024)
    w1pk_h[:, 1024:2048] = w1a_h.reshape(128, 1024)
    # bf16 pack: [identb 128 | w2o 8 | w2a 4 | idx 160]
    bpack_h = np.empty((128, 300), bf)
    bpack_h[:, 0:128] = np.eye(128, dtype=f32).astype(bf)
    bpack_h[:, 128:136] = w2o_h
    bpack_h[:, 136:140] = w2a_h

    # host-side keypoint geometry (all gather indices + seed weights derive
    # from keypoint_coords only)
    kp = np.asarray(keypoint_coords, f32)           # [32, J, 2]
    ix = (kp[..., 0] + 1.0) * 31.5                  # [32, J]
    iy = (kp[..., 1] + 1.0) * 31.5
    x0 = np.floor(ix); y0 = np.floor(iy)
    fx = ix - x0; fy = iy - y0
    bx = np.clip(np.round(ix) - 1.0, 0.0, 61.0)
    by = np.clip(np.round(iy) - 1.0, 0.0, 61.0)
    pos3 = np.arange(3, dtype=f32)
    rowidx = ((by[..., None] + pos3) * 64.0 + bx[..., None])  # [32, J, 3y]
    seedrow = ((y0[..., None] + pos3[:2]) * 64.0 + x0[..., None])  # [32,J,2y]

    in_maps = []
    for m in range(n_cores):
        bs = slice(B * m, B * (m + 1))
        feat_h = np.ascontiguousarray(
            np.asarray(features[bs], f32).transpose(0, 2, 3, 1).reshape(B * HW, C)
        ).astype(bf)
        bpc = bpack_h.copy()
        idxg_h = np.empty((128, 64 + B * 24), np.int16)
        # seed idx: i = y*512 + b*J + j
        sflat = np.empty(2 * J * B, np.int16)
        for y in range(2):
            for b in range(B):
                sflat[y * J * B + b * J : y * J * B + (b + 1) * J] = (
                    seedrow[B * m + b, :, y] + b * HW
                ).astype(np.int16)
        idxg_h[:, 0:64] = wrap(sflat)
        # patch idx per b: i = y*J + j
        for b in range(B):
            flat = (rowidx[B * m + b].T.reshape(3 * J) + b * HW).astype(np.int16)
            idxg_h[:, 64 + b * 24 : 64 + (b + 1) * 24] = wrap(flat)
        # seed weights wsg[x, y, b*J+j] = wx(x)*wy(y)
        fxc = fx[bs].T  # [J, B]
        fyc = fy[bs].T
        wsg_h = np.empty((2, 2, J * B), f32)
        for x in range(2):
            for y in range(2):
                wx = (1.0 - fxc) if x == 0 else fxc
                wy = (1.0 - fyc) if y == 0 else fyc
                wsg_h[x, y] = (wx * wy).T.reshape(J * B)
        # f32 pack: [coef 16 | b1o | b1a | posc 4 | b2o col | b2a col]
        fpack_h = np.zeros((128, 24), f32)
        fpack_h[:, 0:4] = ix[bs].T
        fpack_h[:, 4:8] = iy[bs].T
        fpack_h[:, 8:12] = bx[bs].T
        fpack_h[:, 12:16] = by[bs].T
        fpack_h[:, 16] = np.asarray(b_off1, f32)
        fpack_h[:, 17] = np.asarray(b_att1, f32)
        fpack_h[:, 18:22] = np.arange(4, dtype=f32)[None, :]
        fpack_h[0:8, 22] = np.concatenate([b_off2[0::2], b_off2[1::2]]).astype(f32)
        fpack_h[0:4, 23] = np.asarray(b_att2, f32)
        bpc[:, 140:300] = idxg_h.view(bf)
        in_maps.append({
            "feat": feat_h,
            "wsg": np.broadcast_to(
                wsg_h.reshape(-1).astype(bf)[None, :], (128, 2048)
            ).copy(),
            "bpack": bpc, "fpack": fpack_h, "w1pk": w1pk_h,
        })
    return in_maps


_NC_CACHE = None


def get_nc():
    global _NC_CACHE
    if _NC_CACHE is None:
        _NC_CACHE = build_nc()
    return _NC_CACHE


def kernel(**inputs):
    from concourse.bass_utils import run_bass_kernel_spmd

    n_cores = 8
    nc = get_nc()
    in_maps = prepare_in_maps(**inputs, n_cores=n_cores)
    res = run_bass_kernel_spmd(
        nc, in_maps, core_ids=list(range(n_cores)),
        trace=bool(int(os.environ.get("KERNEL_TRACE", "0") or 0)),
    )
    kernel.last_results = res
    outs = [
        np.asarray(r["out"]).astype(np.float32).reshape(B, J, C)
        for r in res.results
    ]
    return np.concatenate(outs, axis=0)
